# revision 4
# baseline (speedup 1.0000x reference)
"""PixelNCELoss Trainium2 kernel.

Strategy (data-parallel, 8 cores = 4 samples x 2 row-shards):
  - Host: gather anchor features Xq/Xk ([950, 256] per sample) from the
    full feature maps using sample_idx, pre-scale Xq by 1/TEMP.
  - Shard each sample's 950 anchor rows over 2 cores: shard 0 = classes
    0..9 (500 rows), shard 1 = classes 10..18 (450 rows). Columns (the
    950 negatives/positives) are permuted per core so the shard's own
    classes come first in the same order as its rows -- this makes the
    diagonal-block mask structure identical on every core, so a single
    SPMD program serves all 8 cores.
  - Device per core: sim = XqT.T @ XkT (fp32 matmul into PSUM, logits
    pre-scaled), mask own-class block with -2^100 (per-partition scalar
    broadcast adds), row-max, exp (bias=-max) with fused row-sum, add
    the positive term exp(z_ii - max) separately, log -> loss row.
  - Host: reassemble [3800] output.
"""

import numpy as np

TEMP = 0.07
B, C, HW = 4, 256, 128 * 128
NCLS, NV = 19, 50
P = NCLS * NV          # 950
SPLIT = 500            # rows in shard h=0 (classes 0..9); h=1 gets 450
ROWS = (500, 450)
PADR = 512             # padded rows per core (4 tiles of 128)
NT = 4                 # row tiles per core
COL0 = 512             # psum column chunk split
COL1 = P - COL0        # 438
MASKVAL = -(2.0 ** 100)
INV_T = 1.0 / TEMP

_cache = {}


def _tile_classes(t):
    """Local class range covered by row tile t (static, same on all cores)."""
    r0 = t * 128
    cls_lo = r0 // NV
    cls_hi = (r0 + 127) // NV
    return r0, cls_lo, cls_hi


def _build_program():
    import concourse.bacc as bacc
    import concourse.tile as tile
    from concourse import mybir

    f32 = mybir.dt.float32
    AF = mybir.ActivationFunctionType
    ALU = mybir.AluOpType
    AX = mybir.AxisListType

    nc = bacc.Bacc("TRN2", target_bir_lowering=False, debug=False)
    xqT = nc.declare_dram_parameter("xqT", [C, PADR], f32, isOutput=False)
    xkT = nc.declare_dram_parameter("xkT", [C, P], f32, isOutput=False)
    ident = nc.declare_dram_parameter("ident", [128, 128], f32, isOutput=False)
    bsel = nc.declare_dram_parameter("bsel", [128, NT, 4], f32, isOutput=False)
    outp = nc.declare_dram_parameter("out", [128, NT], f32, isOutput=True)

    xqT_r = xqT.rearrange("(kc p) m -> kc p m", p=128)   # [2, 128, 512]
    xkT_r = xkT.rearrange("(kc p) n -> kc p n", p=128)   # [2, 128, 950]

    with tile.TileContext(nc) as tc:
        with (
            tc.tile_pool(name="singles", bufs=1) as singles,
            tc.tile_pool(name="data", bufs=1) as data,
            tc.tile_pool(name="psum", bufs=2, space="PSUM") as psum,
            tc.tile_pool(name="epool", bufs=2) as epool,
            tc.tile_pool(name="scrp", bufs=2) as scrp,
            tc.tile_pool(name="small", bufs=2) as small,
        ):
            ident_sb = singles.tile([128, 128], f32)
            nc.sync.dma_start(out=ident_sb, in_=ident[:, :])
            bsel_sb = singles.tile([128, NT, 4], f32)
            nc.sync.dma_start(out=bsel_sb, in_=bsel[:, :, :])
            loss_sb = singles.tile([128, NT], f32)

            xq_sb = data.tile([128, 2, PADR], f32)
            xk_sb = data.tile([128, 2, P], f32)
            for kc in range(2):
                nc.sync.dma_start(out=xq_sb[:, kc, :], in_=xqT_r[kc])
                # split xk by column chunk too, for finer DMA/compute overlap
                nc.sync.dma_start(out=xk_sb[:, kc, 0:COL0], in_=xkT_r[kc, :, 0:COL0])
                nc.sync.dma_start(out=xk_sb[:, kc, COL0:P], in_=xkT_r[kc, :, COL0:P])

            for t in range(NT):
                r0, cls_lo, cls_hi = _tile_classes(t)
                ps0 = psum.tile([128, COL0], f32, tag="ps0")
                ps1 = psum.tile([128, COL1], f32, tag="ps1")
                for kc in range(2):
                    nc.tensor.matmul(
                        ps0,
                        xq_sb[:, kc, r0:r0 + 128],
                        xk_sb[:, kc, 0:COL0],
                        start=(kc == 0),
                        stop=(kc == 1),
                    )
                for kc in range(2):
                    nc.tensor.matmul(
                        ps1,
                        xq_sb[:, kc, r0:r0 + 128],
                        xk_sb[:, kc, COL0:P],
                        start=(kc == 0),
                        stop=(kc == 1),
                    )

                # positive logit z_ii: diagonal of the [128,128] block at
                # columns r0..r0+128 (always inside chunk 0). Must read
                # BEFORE the mask add destroys the diagonal values.
                scr = scrp.tile([128, 128], f32)
                lpos = small.tile([128, 1], f32, tag="lpos")
                nc.vector.tensor_mul(scr, ps0[:, r0:r0 + 128], ident_sb)
                nc.vector.reduce_sum(lpos, scr, axis=AX.X)

                # mask own-class 50-column blocks in place: ps += bsel
                # (bsel[p,t,s] = MASKVAL if class(p)==cls_lo+s else 0)
                for s in range(cls_hi - cls_lo + 1):
                    c = cls_lo + s
                    lo, hi = c * NV, c * NV + NV
                    sel = bsel_sb[:, t, s:s + 1]
                    if hi <= COL0:
                        nc.vector.tensor_scalar_add(
                            out=ps0[:, lo:hi], in0=ps0[:, lo:hi], scalar1=sel)
                    elif lo >= COL0:
                        nc.vector.tensor_scalar_add(
                            out=ps1[:, lo - COL0:hi - COL0],
                            in0=ps1[:, lo - COL0:hi - COL0], scalar1=sel)
                    else:
                        nc.vector.tensor_scalar_add(
                            out=ps0[:, lo:COL0], in0=ps0[:, lo:COL0], scalar1=sel)
                        nc.vector.tensor_scalar_add(
                            out=ps1[:, 0:hi - COL0],
                            in0=ps1[:, 0:hi - COL0], scalar1=sel)

                # row max over masked logits, then include the positive
                m0 = small.tile([128, 1], f32, tag="m0")
                m1 = small.tile([128, 1], f32, tag="m1")
                nc.vector.reduce_max(m0, ps0[:, :], axis=AX.X)
                nc.vector.reduce_max(m1, ps1[:, :], axis=AX.X)
                mp = small.tile([128, 1], f32, tag="mp")
                nc.vector.scalar_tensor_tensor(
                    out=mp, in0=m0, scalar=lpos, in1=m1,
                    op0=ALU.max, op1=ALU.max)
                negm = small.tile([128, 1], f32, tag="negm")
                nc.vector.tensor_scalar_mul(out=negm, in0=mp, scalar1=-1.0)

                # exp + fused row sums; masked entries underflow to 0
                E0 = epool.tile([128, COL0], f32, tag="E0")
                E1 = epool.tile([128, COL1], f32, tag="E1")
                S0 = small.tile([128, 1], f32, tag="S0")
                S1 = small.tile([128, 1], f32, tag="S1")
                nc.scalar.activation(E0, ps0[:, :], AF.Exp,
                                     bias=negm, scale=1.0, accum_out=S0)
                nc.scalar.activation(E1, ps1[:, :], AF.Exp,
                                     bias=negm, scale=1.0, accum_out=S1)
                e_d = small.tile([128, 1], f32, tag="e_d")
                nc.scalar.activation(e_d, lpos, AF.Exp, bias=negm, scale=1.0)

                Ssum = small.tile([128, 1], f32, tag="Ssum")
                nc.vector.scalar_tensor_tensor(
                    out=Ssum, in0=S0, scalar=e_d, in1=S1,
                    op0=ALU.add, op1=ALU.add)
                logS = small.tile([128, 1], f32, tag="logS")
                nc.scalar.activation(logS, Ssum, AF.Ln)
                # loss = (mp - lpos) + logS
                nc.vector.scalar_tensor_tensor(
                    out=loss_sb[:, t:t + 1], in0=mp, scalar=lpos, in1=logS,
                    op0=ALU.subtract, op1=ALU.add)

            nc.sync.dma_start(out=outp[:, :], in_=loss_sb)

    nc.compile()
    return nc


def _get_program():
    if "nc" not in _cache:
        _cache["nc"] = _build_program()
    return _cache["nc"]


def _host_inputs(feats_q, feats_k, sample_idx):
    """Build the 8 per-core input maps."""
    q = np.ascontiguousarray(feats_q, dtype=np.float32).reshape(B, C, HW)
    k = np.ascontiguousarray(feats_k, dtype=np.float32).reshape(B, C, HW)
    idx = np.asarray(sample_idx).reshape(B, P)

    ident = np.eye(128, dtype=np.float32)
    bsel = np.zeros((128, NT, 4), dtype=np.float32)
    for t in range(NT):
        r0, cls_lo, cls_hi = _tile_classes(t)
        cls_of_p = (r0 + np.arange(128)) // NV
        for s in range(cls_hi - cls_lo + 1):
            bsel[cls_of_p == cls_lo + s, t, s] = MASKVAL

    colperm1 = np.concatenate([np.arange(SPLIT, P), np.arange(0, SPLIT)])

    in_maps = []
    for b in range(B):
        XqT = q[b][:, idx[b]]                     # [C, P]
        XkT = k[b][:, idx[b]]
        for h in range(2):
            r0g = 0 if h == 0 else SPLIT
            nrows = ROWS[h]
            xq = np.zeros((C, PADR), dtype=np.float32)
            xq[:, :nrows] = XqT[:, r0g:r0g + nrows] * np.float32(INV_T)
            xk = XkT if h == 0 else np.ascontiguousarray(XkT[:, colperm1])
            in_maps.append({
                "xqT": xq,
                "xkT": np.ascontiguousarray(xk),
                "ident": ident,
                "bsel": bsel,
            })
    return in_maps


def _assemble(results):
    out = np.zeros((B, P), dtype=np.float32)
    for b in range(B):
        for h in range(2):
            r0g = 0 if h == 0 else SPLIT
            nrows = ROWS[h]
            arr = np.asarray(results[2 * b + h]["out"])  # [128, NT]
            loss = arr.T.reshape(PADR)                   # index t*128+p
            out[b, r0g:r0g + nrows] = loss[:nrows]
    return out.reshape(-1)


def kernel(feats_q, feats_k, sample_idx):
    from concourse.bass_utils import run_bass_kernel_spmd

    nc = _get_program()
    in_maps = _host_inputs(feats_q, feats_k, sample_idx)
    res = run_bass_kernel_spmd(nc, in_maps, list(range(8)))
    return _assemble(res.results)


# revision 10
# speedup vs baseline: 1.2709x; 1.2709x over previous
"""PixelNCELoss Trainium2 kernel.

Strategy (data-parallel, 8 cores = 4 samples x 2 row-shards):
  - Host: gather anchor features Xq/Xk ([950, 256] per sample) from the
    full feature maps using sample_idx, pre-scale Xq by 1/TEMP.
  - Shard each sample's 950 anchor rows over 2 cores: shard 0 = classes
    0..9 (500 rows), shard 1 = classes 10..18 (450 rows). Columns (the
    950 negatives/positives) are permuted per core so the shard's own
    classes come first in the same order as its rows -- this makes the
    diagonal-block mask structure identical on every core, so a single
    SPMD program serves all 8 cores.
  - Device per core: sim = XqT.T @ XkT (fp32 matmul into PSUM, logits
    pre-scaled), mask own-class block with -2^100 (per-partition scalar
    broadcast adds), row-max, exp (bias=-max) with fused row-sum, add
    the positive term exp(z_ii - max) separately, log -> loss row.
  - Host: reassemble [3800] output.
"""

import numpy as np

TEMP = 0.07
B, C, HW = 4, 256, 128 * 128
NCLS, NV = 19, 50
P = NCLS * NV          # 950
SPLIT = 500            # rows in shard h=0 (classes 0..9); h=1 gets 450
ROWS = (500, 450)
PADR = 512             # padded rows per core (4 tiles of 128)
NT = 4                 # row tiles per core
COL0 = 512             # psum column chunk split
COL1 = P - COL0        # 438
MASKVAL = -(2.0 ** 100)
INV_T = 1.0 / TEMP

_cache = {}


def _tile_classes(t):
    """Local class range covered by row tile t (static, same on all cores)."""
    r0 = t * 128
    cls_lo = r0 // NV
    cls_hi = (r0 + 127) // NV
    return r0, cls_lo, cls_hi


def _build_program():
    import concourse.bacc as bacc
    import concourse.tile as tile
    from concourse import mybir

    f32 = mybir.dt.float32
    AF = mybir.ActivationFunctionType
    ALU = mybir.AluOpType
    AX = mybir.AxisListType

    f32r = mybir.dt.float32r
    nc = bacc.Bacc("TRN2", target_bir_lowering=False, debug=False)
    xqT = nc.declare_dram_parameter("xqT", [C, PADR], f32r, isOutput=False)
    xkT = nc.declare_dram_parameter("xkT", [C, P], f32r, isOutput=False)
    ident = nc.declare_dram_parameter("ident", [128, 128], f32, isOutput=False)
    bsel = nc.declare_dram_parameter("bsel", [128, NT, 4], f32, isOutput=False)
    outp = nc.declare_dram_parameter("out", [128, NT], f32, isOutput=True)

    xqT_r = xqT.rearrange("(kc p) m -> kc p m", p=128)   # [2, 128, 512]
    xkT_r = xkT.rearrange("(kc p) n -> kc p n", p=128)   # [2, 128, 950]

    with tile.TileContext(nc) as tc:
        with (
            tc.tile_pool(name="singles", bufs=1) as singles,
            tc.tile_pool(name="data", bufs=1) as data,
            tc.tile_pool(name="psum", bufs=2, space="PSUM") as psum,
            tc.tile_pool(name="epool", bufs=2) as epool,
            tc.tile_pool(name="scrp", bufs=2) as scrp,
            tc.tile_pool(name="small", bufs=2) as small,
        ):
            ident_sb = singles.tile([128, 128], f32)
            nc.sync.dma_start(out=ident_sb, in_=ident[:, :])
            bsel_sb = singles.tile([128, NT, 4], f32)
            nc.sync.dma_start(out=bsel_sb, in_=bsel[:, :, :])
            loss_sb = singles.tile([128, NT], f32)
            # per-tile [128,1] results accumulated as columns, so the
            # ln/exp epilogue runs ONCE on [128,4] (avoids per-tile ACT
            # table thrash between Exp and Ln sets)
            mp4 = singles.tile([128, NT], f32)
            lpos4 = singles.tile([128, NT], f32)
            S4 = singles.tile([128, NT], f32)

            xq_sb = data.tile([128, 2, PADR], f32r)
            xk_sb = data.tile([128, 2, P], f32r)
            for kc in range(2):
                nc.sync.dma_start(out=xq_sb[:, kc, :], in_=xqT_r[kc])
                # split xk by column chunk too, for finer DMA/compute overlap
                nc.sync.dma_start(out=xk_sb[:, kc, 0:COL0], in_=xkT_r[kc, :, 0:COL0])
                nc.sync.dma_start(out=xk_sb[:, kc, COL0:P], in_=xkT_r[kc, :, COL0:P])

            for t in range(NT):
                r0, cls_lo, cls_hi = _tile_classes(t)
                ps0 = psum.tile([128, COL0], f32, tag="ps0")
                ps1 = psum.tile([128, COL1], f32, tag="ps1")
                for kc in range(2):
                    nc.tensor.matmul(
                        ps0,
                        xq_sb[:, kc, r0:r0 + 128],
                        xk_sb[:, kc, 0:COL0],
                        start=(kc == 0),
                        stop=(kc == 1),
                    )
                for kc in range(2):
                    nc.tensor.matmul(
                        ps1,
                        xq_sb[:, kc, r0:r0 + 128],
                        xk_sb[:, kc, COL0:P],
                        start=(kc == 0),
                        stop=(kc == 1),
                    )

                # positive logit z_ii: diagonal of the [128,128] block at
                # columns r0..r0+128 (always inside chunk 0). Must read
                # BEFORE the mask add destroys the diagonal values.
                scr = scrp.tile([128, 128], f32)
                lpos = lpos4[:, t:t + 1]
                nc.vector.tensor_mul(scr, ps0[:, r0:r0 + 128], ident_sb)
                nc.vector.reduce_sum(lpos, scr, axis=AX.X)

                # mask own-class 50-column blocks in place: ps += bsel
                # (bsel[p,t,s] = MASKVAL if class(p)==cls_lo+s else 0)
                for s in range(cls_hi - cls_lo + 1):
                    c = cls_lo + s
                    lo, hi = c * NV, c * NV + NV
                    sel = bsel_sb[:, t, s:s + 1]
                    if hi <= COL0:
                        nc.vector.tensor_scalar_add(
                            out=ps0[:, lo:hi], in0=ps0[:, lo:hi], scalar1=sel)
                    elif lo >= COL0:
                        nc.vector.tensor_scalar_add(
                            out=ps1[:, lo - COL0:hi - COL0],
                            in0=ps1[:, lo - COL0:hi - COL0], scalar1=sel)
                    else:
                        nc.vector.tensor_scalar_add(
                            out=ps0[:, lo:COL0], in0=ps0[:, lo:COL0], scalar1=sel)
                        nc.vector.tensor_scalar_add(
                            out=ps1[:, 0:hi - COL0],
                            in0=ps1[:, 0:hi - COL0], scalar1=sel)

                # row max over masked logits, then include the positive
                m0 = small.tile([128, 1], f32, tag="m0")
                m1 = small.tile([128, 1], f32, tag="m1")
                nc.vector.reduce_max(m0, ps0[:, :], axis=AX.X)
                nc.vector.reduce_max(m1, ps1[:, :], axis=AX.X)
                mp = mp4[:, t:t + 1]
                nc.vector.scalar_tensor_tensor(
                    out=mp, in0=m0, scalar=lpos, in1=m1,
                    op0=ALU.max, op1=ALU.max)
                negm = small.tile([128, 1], f32, tag="negm")
                nc.vector.tensor_scalar_mul(out=negm, in0=mp, scalar1=-1.0)

                # exp + fused row sums; masked entries underflow to 0
                E0 = epool.tile([128, COL0], f32, tag="E0")
                E1 = epool.tile([128, COL1], f32, tag="E1")
                S0 = small.tile([128, 1], f32, tag="S0")
                S1 = small.tile([128, 1], f32, tag="S1")
                nc.scalar.activation(E0, ps0[:, :], AF.Exp,
                                     bias=negm, scale=1.0, accum_out=S0)
                nc.scalar.activation(E1, ps1[:, :], AF.Exp,
                                     bias=negm, scale=1.0, accum_out=S1)
                nc.vector.tensor_add(S4[:, t:t + 1], S0, S1)

            # batched epilogue over [128, NT]: one Exp, one Ln -- exactly
            # two ACT table switches for the whole kernel
            earg = singles.tile([128, NT], f32)
            nc.vector.tensor_sub(earg, lpos4, mp4)          # lpos - mp <= 0
            e_d4 = singles.tile([128, NT], f32)
            nc.scalar.activation(e_d4, earg, AF.Exp, bias=0.0, scale=1.0)
            Sf = singles.tile([128, NT], f32)
            nc.vector.tensor_add(Sf, S4, e_d4)
            logS = singles.tile([128, NT], f32)
            nc.scalar.activation(logS, Sf, AF.Ln)
            # loss = (mp - lpos) + logS = logS - earg
            nc.vector.tensor_sub(loss_sb, logS, earg)

            nc.sync.dma_start(out=outp[:, :], in_=loss_sb)

    nc.compile()
    return nc


def _get_program():
    if "nc" not in _cache:
        _cache["nc"] = _build_program()
    return _cache["nc"]


def _host_inputs(feats_q, feats_k, sample_idx):
    """Build the 8 per-core input maps."""
    q = np.ascontiguousarray(feats_q, dtype=np.float32).reshape(B, C, HW)
    k = np.ascontiguousarray(feats_k, dtype=np.float32).reshape(B, C, HW)
    idx = np.asarray(sample_idx).reshape(B, P)

    ident = np.eye(128, dtype=np.float32)
    bsel = np.zeros((128, NT, 4), dtype=np.float32)
    for t in range(NT):
        r0, cls_lo, cls_hi = _tile_classes(t)
        cls_of_p = (r0 + np.arange(128)) // NV
        for s in range(cls_hi - cls_lo + 1):
            bsel[cls_of_p == cls_lo + s, t, s] = MASKVAL

    colperm1 = np.concatenate([np.arange(SPLIT, P), np.arange(0, SPLIT)])

    in_maps = []
    for b in range(B):
        XqT = q[b][:, idx[b]]                     # [C, P]
        XkT = k[b][:, idx[b]]
        for h in range(2):
            r0g = 0 if h == 0 else SPLIT
            nrows = ROWS[h]
            xq = np.zeros((C, PADR), dtype=np.float32)
            xq[:, :nrows] = XqT[:, r0g:r0g + nrows] * np.float32(INV_T)
            xk = XkT if h == 0 else np.ascontiguousarray(XkT[:, colperm1])
            in_maps.append({
                "xqT": xq,
                "xkT": np.ascontiguousarray(xk),
                "ident": ident,
                "bsel": bsel,
            })
    return in_maps


def _assemble(results):
    out = np.zeros((B, P), dtype=np.float32)
    for b in range(B):
        for h in range(2):
            r0g = 0 if h == 0 else SPLIT
            nrows = ROWS[h]
            arr = np.asarray(results[2 * b + h]["out"])  # [128, NT]
            loss = arr.T.reshape(PADR)                   # index t*128+p
            out[b, r0g:r0g + nrows] = loss[:nrows]
    return out.reshape(-1)


def kernel(feats_q, feats_k, sample_idx):
    from concourse.bass_utils import run_bass_kernel_spmd

    nc = _get_program()
    in_maps = _host_inputs(feats_q, feats_k, sample_idx)
    res = run_bass_kernel_spmd(nc, in_maps, list(range(8)))
    return _assemble(res.results)
